# revision 12
# baseline (speedup 1.0000x reference)
"""Trainium2 Bass kernel for the spatial-attention layer.

Math (reference):
    fp = input_h @ f            [B, N, D]   N = 64*64 = 4096, D = 64
    gp = x @ g                  [B, N, D]
    s  = gp @ fp^T              [B, N, N]
    beta = softmax(s, -1)
    o  = beta @ input_h         [B, N, C2]
    out = gamma * o + x

Distribution: 8 cores, core c handles batch b = c // 2 and query rows
[half*2048, (half+1)*2048) with half = c % 2. Each core sees the full
4096 keys of its batch.

Per-core strategy (v6):
  - All input transposes run on the DMA engines (InstDmaTransposeAnt,
    one batched [128, 2048] fp16 call per 4-tile group) instead of the
    PE; the PE only does projections, scores, denominators and the
    fp8 output matmuls.
  - h/x tiles: one batched f32 DMA per group -> fp16 casts (DVE) ->
    DMA transpose -> fp16 projection matmuls.  h also casts to fp8e4
    [h|h] pair tiles (DVE for pairs 0-11, GpSimd for 12-15).
  - Scores are written to PSUM as fp16: one [128, 4, 512] sps tile
    (2 banks) holds TWO key-tile pairs; a single [128, 2048] exp per
    sps tile (bias -4, fp8e5 out) halves ACT instruction overhead.
  - Denominators: all-ones fp8 DoubleRow matmuls col-packed 4x via
    tile_position (4 concurrent chains on col groups), summed +
    transposed by one K=4 matmul per query tile.
  - Output matmuls are fp8 DoubleRow over key-tile pairs (roofline),
    evacuated by scalar_tensor_tensor directly from PSUM.
"""

import numpy as np

import concourse.bass as bass
import concourse.mybir as mybir
import concourse.tile as tile
from concourse import bacc
from concourse.bass_utils import run_bass_kernel_spmd

F32 = mybir.dt.float32
FP16 = mybir.dt.float16
FP8E4 = mybir.dt.float8e4
FP8E5 = mybir.dt.float8e5
MULT = mybir.AluOpType.mult
ADD = mybir.AluOpType.add
DR = mybir.MatmulPerfMode.DoubleRow

B, W, C, D = 4, 64, 512, 64
N = W * W                  # 4096 spatial positions (keys per batch)
NQ = N // 2                # 2048 queries per core
N_CORES = 8
MT = N // 128              # 32 key tiles
PAIRS = MT // 2            # 16 key-tile pairs
QB = 4                     # query blocks of 512
QT = NQ // 128             # 16 query tiles
HG = MT // 4               # 8 h groups of 4 tiles
XG = QT // 4               # 4 x groups of 4 tiles

EXP_FN = mybir.ActivationFunctionType.Exp


DEN_COLPACK = False     # col-packed 4x denominator chains via tile_position
STRIDED_RHS = True      # projection rhs as strided stgT[:, k::4, :]
STRIDED_DEN_COPY = False  # partition-strided PSUM reads are illegal


def build_nc():
    nc = bacc.Bacc(None)
    xh_d = nc.dram_tensor("xh", [NQ, C], F32, kind="ExternalInput")
    h_d = nc.dram_tensor("h", [N, C], F32, kind="ExternalInput")
    f_d = nc.dram_tensor("f", [C, D], F32, kind="ExternalInput")
    g_d = nc.dram_tensor("g", [C, D], F32, kind="ExternalInput")
    gamma_d = nc.dram_tensor("gamma", [1], F32, kind="ExternalInput")
    out_d = nc.dram_tensor("out", [NQ, C], F32, kind="ExternalOutput")

    with tile.TileContext(nc) as tc:
        with (
            tc.tile_pool(name="consts", bufs=1) as consts,
            tc.tile_pool(name="hf_pool", bufs=3) as hf_pool,
            tc.tile_pool(name="h16_pool", bufs=3) as h16_pool,
            tc.tile_pool(name="stg_pool", bufs=3) as stg_pool,
            tc.tile_pool(name="h8_pool", bufs=PAIRS) as h8_pool,
            tc.tile_pool(name="x_pool", bufs=XG) as x_pool,
            tc.tile_pool(name="p_pool", bufs=4 * PAIRS) as p_pool,
            tc.tile_pool(name="scales", bufs=8) as scales,
            tc.tile_pool(name="outp", bufs=4) as outp,
            tc.tile_pool(name="psA", bufs=2, space="PSUM") as psA,
            tc.tile_pool(name="psB", bufs=2, space="PSUM") as psB,
            tc.tile_pool(name="psC", bufs=1, space="PSUM") as psC,
            tc.tile_pool(name="psD", bufs=1, space="PSUM") as psD,
        ):
            # ---- ACT table preload + constants ----------------------------
            seed = consts.tile([128, 1], F32)
            nc.vector.memset(seed, 0.0)
            seed_out = consts.tile([128, 1], F32)
            nc.scalar.activation(seed_out, seed, EXP_FN)

            exp_bias = consts.tile([128, 1], F32)
            nc.vector.memset(exp_bias, -4.0)

            ones8 = consts.tile([128, 2, 32], FP8E4)
            nc.vector.memset(ones8, 1.0)
            # every DR den output row equals the denominator partial, so the
            # den transpose matmul sums all 32*nk rows scaled by 1/32.
            inv32 = consts.tile([128, 1], F32)
            nc.vector.memset(inv32, 1.0 / 32.0)

            gamma_sb = consts.tile([128, 1], F32)
            nc.sync.dma_start(
                gamma_sb,
                bass.AP(tensor=gamma_d, offset=0, ap=[[0, 128], [1, 1]]),
            )

            # f, g: [512, 64] -> per-k-chunk [128, 128] fp16 tiles with the
            # chunk duplicated on cols 0:64 and 64:128 so fpT/gpT come out
            # duplicated across both partition halves (for row tiling).
            fg_f32 = consts.tile([128, 2, 4, D], F32)
            nc.sync.dma_start(
                fg_f32[:, 0], f_d[:, :].rearrange("(k p) d -> p k d", p=128))
            nc.sync.dma_start(
                fg_f32[:, 1], g_d[:, :].rearrange("(k p) d -> p k d", p=128))
            f_tiles = [consts.tile([128, 2 * D], FP16, name=f"f16_{k}")
                       for k in range(4)]
            g_tiles = [consts.tile([128, 2 * D], FP16, name=f"g16_{k}")
                       for k in range(4)]
            for k in range(4):
                nc.vector.tensor_copy(f_tiles[k][:, 0:D], fg_f32[:, 0, k])
                nc.vector.tensor_copy(f_tiles[k][:, D:2 * D], fg_f32[:, 0, k])
                nc.vector.tensor_copy(g_tiles[k][:, 0:D], fg_f32[:, 1, k])
                nc.vector.tensor_copy(g_tiles[k][:, D:2 * D], fg_f32[:, 1, k])

            # ---- PE warmup: ~4us of matmuls during the DMA ramp ------------
            dummy16 = consts.tile([128, 64], FP16)
            nc.vector.memset(dummy16, 0.0625)
            warm_ps = psB.tile([128, 512], F32, tag="out")
            for _ in range(20):
                nc.tensor.matmul(
                    warm_ps[0:64, 0:64], dummy16, dummy16,
                    start=True, stop=True,
                )

            proj_f = [consts.tile([128, 512], FP16, name=f"projf_{i}")
                      for i in range(HG)]
            proj_g = [consts.tile([128, 512], FP16, name=f"projg_{i}")
                      for i in range(QB)]
            h8_tiles = [h8_pool.tile([128, 2, 512], FP8E4, tag="h8",
                                     name=f"h8_{t}")
                        for t in range(PAIRS)]
            x_grp = [x_pool.tile([128, 4, C], F32, tag="x", name=f"xg_{i}")
                     for i in range(XG)]

            # ---- load/cast/transpose/project one group ---------------------
            def do_group(is_x, gi):
                src = xh_d if is_x else h_d
                tag = "x" if is_x else "h"
                if is_x:
                    grp = x_grp[gi]
                else:
                    grp = hf_pool.tile([128, 4, C], F32, tag="hf",
                                       name=f"hg_{gi}")
                nc.sync.dma_start(
                    grp,
                    src[gi * 512:(gi + 1) * 512, :]
                    .rearrange("(j p) c -> p j c", p=128))
                g16 = h16_pool.tile([128, 4, C], FP16, tag="g16",
                                    name=f"g16_{tag}{gi}")
                for j in range(4):
                    nc.vector.tensor_copy(g16[:, j], grp[:, j])
                if not is_x:
                    # fp8 pair tiles for the output matmul
                    for j in range(4):
                        m = gi * 4 + j
                        h8v = h8_tiles[m // 2][:, m % 2]
                        if m < 24:
                            nc.vector.tensor_copy(h8v, grp[:, j])
                        else:
                            nc.gpsimd.tensor_copy(h8v, grp[:, j])
                stgT = stg_pool.tile([128, 16, 128], FP16, tag="stg",
                                     name=f"stgT_{tag}{gi}")
                nc.sync.dma_start_transpose(
                    stgT, g16.rearrange("p a b -> p (a b)"))
                pp = psD.tile([128, 512], F32, tag="pp")
                wt = g_tiles if is_x else f_tiles
                if STRIDED_RHS:
                    for k in range(4):
                        nc.tensor.matmul(
                            pp, wt[k], stgT[:, k::4, :],
                            start=(k == 0), stop=(k == 3),
                        )
                else:
                    for j in range(4):
                        for k in range(4):
                            nc.tensor.matmul(
                                pp[:, j * 128:(j + 1) * 128],
                                wt[k], stgT[:, 4 * j + k, :],
                                start=(k == 0), stop=(k == 3),
                                skip_group_check=True,
                            )
                dst = proj_g[gi] if is_x else proj_f[gi]
                nc.vector.tensor_copy(dst, pp)

            order = [(True, 0), (False, 0), (False, 1), (True, 1),
                     (False, 2), (False, 3), (True, 2), (False, 4),
                     (False, 5), (True, 3), (False, 6), (False, 7)]
            for is_x, gi in order:
                do_group(is_x, gi)

            # ---- scores + exp for all blocks ------------------------------
            p_tiles = [[None] * PAIRS for _ in range(QB)]
            for nb in range(QB):
                q0 = proj_g[nb][0:64, :]
                q1 = proj_g[nb][64:128, :]
                for t in range(PAIRS):
                    m0, m1 = 2 * t, 2 * t + 1
                    sps = psA.tile([128, 2, 512], F32, tag="psA")
                    nc.tensor.matmul(
                        sps[:, 0],
                        proj_f[m0 // 4][0:64,
                                        (m0 % 4) * 128:(m0 % 4 + 1) * 128],
                        q0, start=True, stop=True, skip_group_check=True,
                    )
                    nc.tensor.matmul(
                        sps[:, 1],
                        proj_f[m1 // 4][64:128,
                                        (m1 % 4) * 128:(m1 % 4 + 1) * 128],
                        q1, start=True, stop=True, skip_group_check=True,
                    )
                    pt = p_pool.tile([128, 2, 512], FP8E5, tag="p")
                    p_tiles[nb][t] = pt
                    # bias -4 keeps exp(s) within fp8e5 range; cancels in
                    # the softmax normalization.
                    nc.scalar.activation(pt[:, :, :], sps[:, :, :], EXP_FN,
                                         bias=exp_bias)

            def p_pair(nb, t):
                return p_tiles[nb][t]

            # ---- denominators: col-packed DR chains ------------------------
            sc4s = []
            for nb in range(QB):
                den_ps = psC.tile([128, 512], F32, tag="den")
                if DEN_COLPACK:
                    for s in range(4):
                        for j in range(4):
                            t = 4 * s + j
                            nc.tensor.matmul(
                                den_ps[32 * j:32 * (j + 1), :],
                                ones8, p_pair(nb, t),
                                start=(s == 0), stop=(s == 3),
                                perf_mode=DR, skip_group_check=True,
                                tile_position=(0, 32 * j),
                            )
                    nk = 4
                else:
                    for t in range(PAIRS):
                        nc.tensor.matmul(
                            den_ps[0:32, :],
                            ones8, p_pair(nb, t),
                            start=(t == 0), stop=(t == PAIRS - 1),
                            perf_mode=DR,
                        )
                    nk = 1
                nr = 32 * nk
                den_sb = scales.tile([nr, 512], F32, tag="den",
                                     name=f"den_{nb}")
                nc.vector.tensor_copy(den_sb, den_ps[0:nr, :])
                den_t = psD.tile([128, 512], F32, tag="pp")
                for nt in range(4):
                    nc.tensor.matmul(
                        den_t[:, nt:nt + 1],
                        den_sb[:, nt * 128:(nt + 1) * 128],
                        inv32[0:nr], start=True, stop=True,
                    )
                sc4 = scales.tile([128, 4], F32, tag="scale",
                                  name=f"sc4_{nb}")
                sc4s.append(sc4)
                nc.vector.reciprocal(sc4, den_t[:, 0:4])
                nc.vector.tensor_scalar_mul(sc4, sc4, gamma_sb)

            # ---- output matmuls for all blocks -----------------------------
            for nb in range(QB):
                sc4 = sc4s[nb]
                for nt in range(4):
                    ops = psB.tile([128, 512], F32, tag="out")
                    for t in range(PAIRS):
                        nc.tensor.matmul(
                            ops,
                            p_pair(nb, t)[:, :, nt * 128:(nt + 1) * 128],
                            h8_tiles[t][:, :, 0:512],
                            start=(t == 0), stop=(t == PAIRS - 1),
                            perf_mode=DR,
                        )
                    n_idx = nb * 4 + nt
                    xres = x_grp[n_idx // 4][:, n_idx % 4]
                    out_sb = outp.tile([128, C], F32, tag="out")
                    nc.vector.scalar_tensor_tensor(
                        out_sb[:, 0:256], ops[:, 0:256], sc4[:, nt:nt + 1],
                        xres[:, 0:256], op0=MULT, op1=ADD)
                    nc.vector.scalar_tensor_tensor(
                        out_sb[:, 256:512], ops[:, 256:512], sc4[:, nt:nt + 1],
                        xres[:, 256:512], op0=MULT, op1=ADD)
                    nc.sync.dma_start(
                        out_d[n_idx * 128:(n_idx + 1) * 128, :], out_sb)

    nc.finalize()
    return nc


_NC_CACHE = None


def make_in_maps(x, input_h, f, g, gamma):
    x = np.asarray(x, dtype=np.float32)
    input_h = np.asarray(input_h, dtype=np.float32)
    f2 = np.ascontiguousarray(np.asarray(f, dtype=np.float32).reshape(C, D))
    g2 = np.ascontiguousarray(np.asarray(g, dtype=np.float32).reshape(C, D))
    gam = np.ascontiguousarray(np.asarray(gamma, dtype=np.float32).reshape(1))

    x_flat = x.reshape(B, N, C)
    h_flat = input_h.reshape(B, N, C)

    in_maps = []
    for c in range(N_CORES):
        b, half = c // 2, c % 2
        in_maps.append({
            "xh": np.ascontiguousarray(x_flat[b, half * NQ:(half + 1) * NQ]),
            "h": np.ascontiguousarray(h_flat[b]),
            "f": f2,
            "g": g2,
            "gamma": gam,
        })
    return in_maps


def kernel(x, input_h, f, g, gamma):
    global _NC_CACHE
    in_maps = make_in_maps(x, input_h, f, g, gamma)
    if _NC_CACHE is None:
        _NC_CACHE = build_nc()
    res = run_bass_kernel_spmd(_NC_CACHE, in_maps, core_ids=list(range(N_CORES)))

    out = np.empty((B, N, C), dtype=np.float32)
    for c in range(N_CORES):
        b, half = c // 2, c % 2
        out[b, half * NQ:(half + 1) * NQ] = res.results[c]["out"]
    return out.reshape(B, W, W, C)
